# revision 10
# baseline (speedup 1.0000x reference)
"""Additive (Bahdanau) attention for Trainium2, 8 cores — sine-feature version v2.

Reference (B=4, L=1024, D=512, U=64):
    k = x @ Wx; q = x @ Wt
    e = exp(sum_u Wa_u tanh(q_iu + k_ju + bt_u) + ba)
    v = (e / sum_j e) @ x

tanh(s) ~ sum_m c_m sin(w_m s) (M=4 fitted sines), and
sin(w(q+k)) = sin(wq)cos(wk) + cos(wq)sin(wk), so the [L, L, U] tanh
reduction becomes dense [NQ, 2MU] x [2MU, L] bf16 matmuls over trig features.

Range reduction per sine (ACT Sin is valid on [-pi, pi] only):
  k_int = round(q/P_m) on the PE via TWO chained matmuls: (diag(1/P) +
  MAGIC row) then (-MAGIC row) — the fp32 psum write between them rounds
  the angle (MAGIC = 1.5*2^23). No per-lane offset rows (in-matmul
  constant-row offsets are silently absorbed by the chunked psum write).
  r = q - P*k_int via one DVE/Pool scalar_tensor_tensor.
  cos lanes use evenness: cos(w r) = sin(pi/2 - w|r|): one fused
  abs-negate (abs_max 0, mult -1) on the cos half, then Sin with a
  per-partition bias (0 | pi/2). bt folds into the projection copies
  via per-partition bias adds, so no bt anywhere downstream.

DMA: x ships twice (xt for proj/features, xbd for AV) but xt's four
quarters go first on four separate rings; xbd halves are issued after,
so the projection start is gated only by xt + compact weights (~1.13MB).
Output v is written per-128-query chunk on four rings as normalize
completes.

Sharding: core c -> batch c//2, query half c%2 (512 queries, all 1024
keys); no cross-core communication.

Measured baseline v1 (M=5, 3-mm rounding, single-ring output): ~61-64us.
"""

import numpy as np
import concourse.bass as bass
import concourse.mybir as mybir
import concourse.tile as tile
from concourse import bacc
from concourse.bass_utils import run_bass_kernel_spmd

F32 = mybir.dt.float32
BF16 = mybir.dt.bfloat16
Act = mybir.ActivationFunctionType
Alu = mybir.AluOpType

B, L, D, U = 4, 1024, 512, 64
NCORES = 8
NQ = L // 2
NG = L // 128   # key blocks (8)
NI = NQ // 128  # query chunks (4)
DC = D // 128   # contraction chunks (4)
MAGIC = 12582912.0  # 1.5*2^23
TWO_PI = 2.0 * np.pi
M = 4  # sine terms

# periods quantized to 16-bit mantissa (P*k_int stays fp32-exact), w = 2pi/P
PS = [20.5068359375, 6.78369140625, 4.02716064453125, 2.73828125]
WS = [TWO_PI / p for p in PS]
CS = [1.2281112999178767, 0.30912492837845695, 0.11309364882322426,
      0.047495207121532074]
HALF_PI = float(np.pi / 2)

_cached = {}


def _build():
    if "nc" in _cached:
        return _cached["nc"]
    nc = bacc.Bacc("TRN2", target_bir_lowering=False, debug=False, num_devices=NCORES)

    xt = nc.dram_tensor("xt", [128, DC, L], BF16, kind="ExternalInput").ap()
    xbd = nc.dram_tensor("xbd", [128, NG, D], BF16, kind="ExternalInput").ap()
    wtd = nc.dram_tensor("wtd", [128, DC, 64], BF16, kind="ExternalInput").ap()
    wxd = nc.dram_tensor("wxd", [128, DC, 64], BF16, kind="ExternalInput").ap()
    angw = nc.dram_tensor("angw", [128, M, 128], BF16, kind="ExternalInput").ap()
    wamp = nc.dram_tensor("wamp", [128, M], F32, kind="ExternalInput").ap()
    btcol = nc.dram_tensor("btcol", [128, 1], F32, kind="ExternalInput").ap()
    bac = nc.dram_tensor("bac", [128, 1], F32, kind="ExternalInput").ap()
    vout = nc.dram_tensor("v_out", [NQ, D], F32, kind="ExternalOutput").ap()

    from contextlib import ExitStack

    with tile.TileContext(nc) as tc, ExitStack() as ctx:
        const = ctx.enter_context(tc.tile_pool(name="const", bufs=1))
        # constants / weights
        wtd_sb = const.tile([128, DC, 128], BF16, tag="wtd")
        wxd_sb = const.tile([128, DC, 128], BF16, tag="wxd")
        angw_sb = const.tile([128, M, 128], BF16, tag="angw")
        ones1_sb = const.tile([128, 512], BF16, tag="ones1")
        onesd_sb = const.tile([128, 8], BF16, tag="onesd")
        wamp_sb = const.tile([128, M], F32, tag="wamp")
        btcol_sb = const.tile([128, 1], F32, tag="btcol")
        qsb_sb = const.tile([128, 1], F32, tag="qsb")  # Q Sin bias: 0 | pi/2
        ksb_sb = const.tile([128, 1], F32, tag="ksb")  # K Sin bias: pi/2 | 0
        bac_sb = const.tile([128, 1], F32, tag="bac")
        warm_in = const.tile([128, 1], F32, tag="warm_in")
        warm_out = const.tile([128, 1], F32, tag="warm_out")
        wdum_sb = const.tile([128, 128], BF16, tag="wdum")
        # data
        xt_sb = [
            const.tile([128, DC, 256], BF16, tag=f"xtq{qq}", name=f"xtq{qq}")
            for qq in range(4)
        ]
        xb_sb = [
            const.tile([128, D], BF16, tag=f"xb{g}", name=f"xb{g}")
            for g in range(NG)
        ]
        qdup_sb = const.tile([128, NQ], F32, tag="qdup")
        qaug_sb = const.tile([128, NQ], BF16, tag="qaug")
        kdup_sb = const.tile([128, L], F32, tag="kdup")
        kaug_sb = const.tile([128, L], BF16, tag="kaug")
        qf_sb = const.tile([128, M, NQ], BF16, tag="qf")
        qfa_sb = const.tile([128, M, NQ], BF16, tag="qfa")
        kf_sb = const.tile([128, M, L], BF16, tag="kf")
        et_sb = const.tile([128, NG, NQ], BF16, tag="et")

        # ---------------- memsets / ACT table preload ----------------
        nc.vector.memset(warm_in[:], 0.25)
        nc.scalar.activation(warm_out[:], warm_in[:], Act.Sin)
        nc.vector.memset(wdum_sb[:], 0.00390625)
        nc.vector.memset(ones1_sb[:], 1.0)
        nc.gpsimd.memset(onesd_sb[:], 1.0)
        nc.gpsimd.memset(qsb_sb[0:64, :], 0.0)
        nc.gpsimd.memset(qsb_sb[64:128, :], HALF_PI)
        nc.gpsimd.memset(ksb_sb[0:64, :], HALF_PI)
        nc.gpsimd.memset(ksb_sb[64:128, :], 0.0)
        nc.vector.memset(qaug_sb[64:128, :], 1.0)
        nc.vector.memset(kaug_sb[64:128, :], 1.0)

        # ---------------- DMAs ----------------
        # xt quarters on four rings first (projection-critical), compact
        # weights on scalar; xbd halves after xt (needed only at AV).
        nc.sync.dma_start(out=xt_sb[0][:], in_=xt[:, :, 0:256])
        nc.gpsimd.dma_start(out=xt_sb[1][:], in_=xt[:, :, 256:512])
        nc.sync.dma_start(out=xt_sb[2][:], in_=xt[:, :, 512:768])
        nc.gpsimd.dma_start(out=xt_sb[3][:], in_=xt[:, :, 768:1024])
        nc.scalar.dma_start(out=wtd_sb[:, :, 0:64], in_=wtd[:])
        nc.scalar.dma_start(out=wxd_sb[:, :, 0:64], in_=wxd[:])
        nc.scalar.dma_start(out=angw_sb[:], in_=angw[:])
        nc.scalar.dma_start(out=wamp_sb[:], in_=wamp[:])
        nc.scalar.dma_start(out=btcol_sb[:], in_=btcol[:])
        nc.scalar.dma_start(out=bac_sb[:], in_=bac[:])
        for g in range(4):
            nc.sync.dma_start(out=xb_sb[g][:], in_=xbd[:, g, :])
        for g in range(4, NG):
            nc.gpsimd.dma_start(out=xb_sb[g][:], in_=xbd[:, g, :])

        # weight dup: cols 64-127 = cols 0-63 (stationaries feed both halves)
        nc.vector.tensor_copy(wtd_sb[:, :, 64:128], wtd_sb[:, :, 0:64])
        nc.vector.tensor_copy(wxd_sb[:, :, 64:128], wxd_sb[:, :, 0:64])

        # PE warmup burst: pulls the HAM clock gate to K=8/8 during the
        # xt DMA window so projections run at 2.4 GHz
        warm_ctx = tc.tile_pool(name="warm_ps", bufs=1, space="PSUM")
        warm_pool = warm_ctx.__enter__()
        wt_ps = warm_pool.tile([128, 512], F32, tag="wt_ps")
        for _ in range(14):
            nc.tensor.matmul(wt_ps[:], wdum_sb[:], ones1_sb[:],
                             start=True, stop=True)

        # ---------------- projections ----------------
        with tc.tile_pool(name="proj_ps", bufs=1, space="PSUM") as pps:
            qd_ps = pps.tile([128, NQ], F32, tag="qd_ps")
            kd_ps = pps.tile([128, L], F32, tag="kd_ps")
            for qq in range(2):  # query half = quarters 0,1 (host-permuted)
                sl = slice(qq * 256, qq * 256 + 256)
                for c in range(DC):
                    nc.tensor.matmul(
                        qd_ps[:, sl], wtd_sb[:, c, :], xt_sb[qq][:, c, :],
                        start=(c == 0), stop=(c == DC - 1),
                    )
            # bt folds in at the copy: qdup/qaug hold q+bt
            nc.vector.tensor_scalar(
                qaug_sb[0:64, :], qd_ps[0:64, :], btcol_sb[0:64, 0:1], None,
                Alu.add,
            )
            nc.scalar.activation(
                qdup_sb[:], qd_ps[:], Act.Identity, bias=btcol_sb[:, 0:1]
            )
            for qq in range(4):
                sl = slice(qq * 256, qq * 256 + 256)
                for c in range(DC):
                    nc.tensor.matmul(
                        kd_ps[:, sl], wxd_sb[:, c, :], xt_sb[qq][:, c, :],
                        start=(c == 0), stop=(c == DC - 1),
                    )
                if qq == 1:
                    nc.vector.tensor_copy(
                        kaug_sb[0:64, 0:512], kd_ps[0:64, 0:512]
                    )
                    nc.scalar.copy(kdup_sb[:, 0:512], kd_ps[:, 0:512])
            nc.vector.tensor_copy(kaug_sb[0:64, 512:1024], kd_ps[0:64, 512:1024])
            nc.scalar.copy(kdup_sb[:, 512:1024], kd_ps[:, 512:1024])

        warm_ctx.__exit__(None, None, None)

        # ---------------- trig features ----------------
        with (
            tc.tile_pool(name="aq_ps", bufs=2, space="PSUM") as aqp,
            tc.tile_pool(name="ak_ps", bufs=2, space="PSUM") as akp,
            tc.tile_pool(name="rq_sb", bufs=2) as rqp,
            tc.tile_pool(name="rk_sb", bufs=2) as rkp,
        ):
            # m=0: no range reduction (|q| < P0/2)
            nc.scalar.activation(
                qf_sb[:, 0, :], qdup_sb[:], Act.Sin,
                bias=qsb_sb[:, 0:1], scale=float(WS[0]),
            )
            nc.vector.tensor_scalar_mul(
                qfa_sb[:, 0, :], qf_sb[:, 0, :], wamp_sb[:, 0:1]
            )
            nc.scalar.activation(
                kf_sb[:, 0, :], kdup_sb[:], Act.Sin,
                bias=ksb_sb[:, 0:1], scale=float(WS[0]),
            )
            for m in range(1, M):
                negp = float(-PS[m])
                w = float(WS[m])
                # Q side
                aq = aqp.tile([128, NQ], F32, tag="aq", name="aq")
                nc.tensor.matmul(aq[:], angw_sb[:, m - 1, :], qaug_sb[:],
                                 start=True, stop=False)
                nc.tensor.matmul(aq[:], angw_sb[:, M - 1, :], qaug_sb[:],
                                 start=False, stop=True)
                rq = rqp.tile([128, NQ], F32, tag="rq", name="rq")
                nc.vector.scalar_tensor_tensor(
                    rq[:], aq[:], negp, qdup_sb[:], Alu.mult, Alu.add
                )
                # cos half (rows 64-127): r <- -|r| = min(-r, r)
                nc.vector.scalar_tensor_tensor(
                    rq[64:128, :], rq[64:128, :], -1.0, rq[64:128, :],
                    Alu.mult, Alu.min,
                )
                nc.scalar.activation(
                    qf_sb[:, m, :], rq[:], Act.Sin,
                    bias=qsb_sb[:, 0:1], scale=w,
                )
                nc.vector.tensor_scalar_mul(
                    qfa_sb[:, m, :], qf_sb[:, m, :], wamp_sb[:, m:m + 1]
                )
                # K side
                ak = akp.tile([128, L], F32, tag="ak", name="ak")
                for half in range(2):
                    sl = slice(half * 512, half * 512 + 512)
                    nc.tensor.matmul(ak[:, sl], angw_sb[:, m - 1, :],
                                     kaug_sb[:, sl], start=True, stop=False)
                    nc.tensor.matmul(ak[:, sl], angw_sb[:, M - 1, :], kaug_sb[:, sl],
                                     start=False, stop=True)
                rk = rkp.tile([128, L], F32, tag="rk", name="rk")
                nc.vector.scalar_tensor_tensor(
                    rk[:], ak[:], negp, kdup_sb[:], Alu.mult, Alu.add
                )
                # cos half for K is rows 0-63: r <- -|r| = min(-r, r)
                nc.vector.scalar_tensor_tensor(
                    rk[0:64, :], rk[0:64, :], -1.0, rk[0:64, :],
                    Alu.mult, Alu.min,
                )
                nc.scalar.activation(
                    kf_sb[:, m, :], rk[:], Act.Sin,
                    bias=ksb_sb[:, 0:1], scale=w,
                )

        # ---------------- scores / exp / AV ----------------
        sc_pool = ctx.enter_context(tc.tile_pool(name="sc", bufs=2, space="PSUM"))
        v_pool = ctx.enter_context(tc.tile_pool(name="vps", bufs=1, space="PSUM"))
        vo_pool = ctx.enter_context(tc.tile_pool(name="vo", bufs=1))
        v_tiles = [
            v_pool.tile([128, D], F32, tag=f"v{ic}", name=f"v{ic}")
            for ic in range(NI)
        ]
        den_ps = v_pool.tile([128, NI, 8], F32, tag="den")

        for g in range(NG):
            sc = sc_pool.tile([128, NQ], F32, tag="sc", name="sc")
            gsl = slice(g * 128, (g + 1) * 128)
            for m in range(M):
                nc.tensor.matmul(
                    sc[:], kf_sb[:, m, gsl], qfa_sb[:, m, :],
                    start=(m == 0), stop=(m == M - 1),
                )
            nc.scalar.activation(
                et_sb[:, g, :], sc[:], Act.Exp, bias=bac_sb[:, 0:1]
            )
            for ic in range(NI):
                isl = slice(ic * 128, (ic + 1) * 128)
                nc.tensor.matmul(
                    v_tiles[ic][:], et_sb[:, g, isl], xb_sb[g][:],
                    start=(g == 0), stop=(g == NG - 1),
                )
                nc.tensor.matmul(
                    den_ps[:, ic, :], et_sb[:, g, isl], onesd_sb[:],
                    # single start/stop across the interleaved ic ranges
                    start=(g == 0 and ic == 0),
                    stop=(g == NG - 1 and ic == NI - 1),
                )

        # ---------------- normalize + out ----------------
        rcol_sb = const.tile([128, NI], F32, tag="rcol")
        v_sb = vo_pool.tile([128, NI, D], F32, tag="vsb", name="v_sb")
        vout_r = vout.rearrange("(ic p) d -> p ic d", p=128)
        out_rings = (nc.sync, nc.gpsimd, nc.sync, nc.gpsimd)
        for ic in range(NI):
            nc.vector.reciprocal(rcol_sb[:, ic:ic + 1], den_ps[:, ic, 0:1])
            # alternate mul engines so the four muls pipeline two-wide
            if ic % 2 == 0:
                nc.scalar.mul(v_sb[:, ic, :], v_tiles[ic][:],
                              rcol_sb[:, ic:ic + 1])
            else:
                nc.vector.tensor_scalar_mul(v_sb[:, ic, :], v_tiles[ic][:],
                                            rcol_sb[:, ic:ic + 1])
            out_rings[ic].dma_start(out=vout_r[:, ic:ic + 1, :],
                                    in_=v_sb[:, ic:ic + 1, :])

    nc.compile()
    _cached["nc"] = nc
    return nc


def _to_bf16(a):
    import ml_dtypes

    return np.asarray(a, dtype=np.float32).astype(ml_dtypes.bfloat16)


def _host_prep(x, Wx, Wt, bt, Wa, ba):
    x = np.ascontiguousarray(x, dtype=np.float32)
    Wx = np.asarray(Wx, dtype=np.float32)
    Wt = np.asarray(Wt, dtype=np.float32)
    bt = np.asarray(bt, dtype=np.float32).reshape(U)
    Wa = np.asarray(Wa, dtype=np.float32).reshape(U)
    ba = np.asarray(ba, dtype=np.float32).reshape(1)

    # compact weights: wtd[p, c, u] = Wt[128c+p, u]
    wtd = np.empty((128, DC, 64), dtype=np.float32)
    wxd = np.empty((128, DC, 64), dtype=np.float32)
    for c in range(DC):
        wtd[:, c, :] = Wt[128 * c:128 * (c + 1), :]
        wxd[:, c, :] = Wx[128 * c:128 * (c + 1), :]

    # rounding stationaries: diag(1/P_m) dup + MAGIC row 80
    angs = np.zeros((128, M, 128), dtype=np.float32)
    for m in range(1, M):
        invp = 1.0 / PS[m]
        for u in range(U):
            angs[u, m - 1, u] = invp
            angs[u, m - 1, 64 + u] = invp
        angs[80, m - 1, :] = MAGIC
    angs[80, M - 1, :] = -MAGIC

    wamp = np.empty((128, M), dtype=np.float32)
    for m in range(M):
        wamp[:64, m] = CS[m] * Wa
        wamp[64:, m] = CS[m] * Wa
    btc = np.empty((128, 1), dtype=np.float32)
    btc[:64, 0] = bt
    btc[64:, 0] = bt
    bac = np.full((128, 1), ba[0], dtype=np.float32)

    shared = {
        "wtd": _to_bf16(wtd), "wxd": _to_bf16(wxd),
        "angw": _to_bf16(angs),
        "wamp": wamp, "btcol": btc, "bac": bac,
    }

    in_maps = []
    for cid in range(NCORES):
        b, h = cid // 2, cid % 2
        # xt[p, c, j] = x[b, j, 128c+p]; quarters permuted so the core's
        # query half occupies quarters 0,1
        xT = x[b].T.reshape(DC, 128, L).transpose(1, 0, 2)  # [128, DC, L]
        xr = x[b]
        if h == 1:
            xT = np.concatenate([xT[:, :, 512:], xT[:, :, :512]], axis=2)
            xr = np.concatenate([xr[512:], xr[:512]], axis=0)
        xbv = xr.reshape(NG, 128, D).transpose(1, 0, 2)  # [128, NG, D]
        m_ = dict(shared)
        m_["xt"] = _to_bf16(np.ascontiguousarray(xT))
        m_["xbd"] = _to_bf16(np.ascontiguousarray(xbv))
        in_maps.append(m_)
    return in_maps


def kernel(x, Wx, Wt, bt, Wa, ba):
    nc = _build()
    in_maps = _host_prep(x, Wx, Wt, bt, Wa, ba)
    res = run_bass_kernel_spmd(nc, in_maps, core_ids=list(range(NCORES)))
    out = np.empty((B, L, D), dtype=np.float32)
    for cid in range(NCORES):
        b, h = cid // 2, cid % 2
        out[b, h * NQ:(h + 1) * NQ, :] = res.results[cid]["v_out"]
    return out


if __name__ == "__main__":
    rng = np.random.default_rng(0)
    x = rng.standard_normal((B, L, D), dtype=np.float32)
    Wx = (rng.standard_normal((D, U), dtype=np.float32) * 0.06).astype(np.float32)
    Wt = (rng.standard_normal((D, U), dtype=np.float32) * 0.06).astype(np.float32)
    bt = np.zeros(U, dtype=np.float32)
    Wa = (rng.standard_normal((U, 1), dtype=np.float32) * 0.17).astype(np.float32)
    ba = np.zeros(1, dtype=np.float32)
    v = kernel(x=x, Wx=Wx, Wt=Wt, bt=bt, Wa=Wa, ba=ba)
    print("kernel ran, out shape", v.shape)


# revision 15
# speedup vs baseline: 1.1298x; 1.1298x over previous
"""Additive (Bahdanau) attention for Trainium2, 8 cores — sine-feature v3.

Reference (B=4, L=1024, D=512, U=64):
    k = x @ Wx; q = x @ Wt
    e = exp(sum_u Wa_u tanh(q_iu + k_ju + bt_u) + ba)
    v = (e / sum_j e) @ x

tanh(s) ~ sum_m c_m sin(w_m s) (M=4 fitted sines, end-to-end rel err
~1.0e-2), and sin(w(q+k)) = sin(wq)cos(wk) + cos(wq)sin(wk), so the
[L, L, U] tanh reduction becomes dense bf16 matmuls over trig features.

Range reduction per sine m>=1 (ACT Sin valid on [-pi, pi]): 3 chained
matmuls per tile: (diag(1/P)+offset row) -> (+MAGIC/128 rows) ->
(-MAGIC/128 rows); the fp32 psum write between chained matmuls rounds
the angle (MAGIC trick). Offset rows carry bt/P and the +1/4-turn cos
shift. r = q - P*k via one DVE STT reading the projection psum
directly (no psum->sbuf f32 copies at all); Sin bias re-adds w*bt and
the pi/2 cos phase per partition.

DMA: xt quarters 0..3 on sync/gpsimd rings only; compact weights then
all of xbd on the scalar ring (xbd is needed only at the AV stage).
Output v per-128-query chunk on sync/gpsimd as normalize completes.

Sharding: core c -> batch c//2, query half c%2; no cross-core traffic.
"""

import numpy as np
import concourse.bass as bass
import concourse.mybir as mybir
import concourse.tile as tile
from concourse import bacc
from concourse.bass_utils import run_bass_kernel_spmd

F32 = mybir.dt.float32
BF16 = mybir.dt.bfloat16
Act = mybir.ActivationFunctionType
Alu = mybir.AluOpType

B, L, D, U = 4, 1024, 512, 64
NCORES = 8
NQ = L // 2
NG = L // 128   # key blocks (8)
NI = NQ // 128  # query chunks (4)
DC = D // 128   # contraction chunks (4)
MAGIC = 12582912.0  # 1.5*2^23
TWO_PI = 2.0 * np.pi
M = 4  # sine terms

# periods quantized to 16-bit mantissa (P*k_int stays fp32-exact), w = 2pi/P
PS = [20.5068359375, 6.78369140625, 4.02716064453125, 2.73828125]
WS = [TWO_PI / p for p in PS]
CS = [1.2281112999178767, 0.30912492837845695, 0.11309364882322426,
      0.047495207121532074]
HALF_PI = float(np.pi / 2)
NWARM = 6

_cached = {}


def _build():
    if "nc" in _cached:
        return _cached["nc"]
    nc = bacc.Bacc("TRN2", target_bir_lowering=False, debug=False, num_devices=NCORES)

    xt = nc.dram_tensor("xt", [128, DC, L], BF16, kind="ExternalInput").ap()
    xbd = nc.dram_tensor("xbd", [128, NG, D], BF16, kind="ExternalInput").ap()
    wtd = nc.dram_tensor("wtd", [128, DC, 64], BF16, kind="ExternalInput").ap()
    wxd = nc.dram_tensor("wxd", [128, DC, 64], BF16, kind="ExternalInput").ap()
    # slots: 0..2 = wangq m=1..3, 3..5 = wangk m=1..3, 6 = +mrow, 7 = -mrow
    angw = nc.dram_tensor("angw", [128, 8, 128], BF16, kind="ExternalInput").ap()
    wamp = nc.dram_tensor("wamp", [128, M], F32, kind="ExternalInput").ap()
    qsbm = nc.dram_tensor("qsbm", [128, M], F32, kind="ExternalInput").ap()
    btcol = nc.dram_tensor("btcol", [128, 1], F32, kind="ExternalInput").ap()
    bac = nc.dram_tensor("bac", [128, 1], F32, kind="ExternalInput").ap()
    vout = nc.dram_tensor("v_out", [NQ, D], F32, kind="ExternalOutput").ap()

    from contextlib import ExitStack

    with tile.TileContext(nc) as tc, ExitStack() as ctx:
        const = ctx.enter_context(tc.tile_pool(name="const", bufs=1))
        wtd_sb = const.tile([128, DC, 128], BF16, tag="wtd")
        wxd_sb = const.tile([128, DC, 128], BF16, tag="wxd")
        angw_sb = const.tile([128, 8, 128], BF16, tag="angw")
        ones1_sb = const.tile([128, 512], BF16, tag="ones1")
        onesd_sb = const.tile([128, 8], BF16, tag="onesd")
        wamp_sb = const.tile([128, M], F32, tag="wamp")
        qsbm_sb = const.tile([128, M], F32, tag="qsbm")
        btcol_sb = const.tile([128, 1], F32, tag="btcol")
        ksb_sb = const.tile([128, 1], F32, tag="ksb")  # K Sin bias: pi/2 | 0
        bac_sb = const.tile([128, 1], F32, tag="bac")
        warm_in = const.tile([128, 1], F32, tag="warm_in")
        warm_out = const.tile([128, 1], F32, tag="warm_out")
        wdum_sb = const.tile([128, 128], BF16, tag="wdum")
        xt_sb = [
            const.tile([128, DC, 256], BF16, tag=f"xtq{qq}", name=f"xtq{qq}")
            for qq in range(4)
        ]
        xb_sb = [
            const.tile([128, D], BF16, tag=f"xb{g}", name=f"xb{g}")
            for g in range(NG)
        ]
        qaug_sb = const.tile([128, NQ], BF16, tag="qaug")
        kaug_sb = const.tile([128, L], BF16, tag="kaug")
        qdup_sb = const.tile([128, NQ], F32, tag="qdup")
        kdup_sb = const.tile([128, L], F32, tag="kdup")
        qf_sb = const.tile([128, M, NQ], BF16, tag="qf")
        qfa_sb = const.tile([128, M, NQ], BF16, tag="qfa")
        kf_sb = const.tile([128, M, L], BF16, tag="kf")
        et_sb = const.tile([128, NG, NQ], BF16, tag="et")

        # ---------------- memsets / ACT table preload ----------------
        nc.vector.memset(warm_in[:], 0.25)
        nc.scalar.activation(warm_out[:], warm_in[:], Act.Sin)
        nc.vector.memset(wdum_sb[:], 0.00390625)
        nc.vector.memset(ones1_sb[:], 1.0)
        nc.gpsimd.memset(onesd_sb[:], 1.0)
        nc.gpsimd.memset(ksb_sb[0:64, :], HALF_PI)
        nc.gpsimd.memset(ksb_sb[64:128, :], 0.0)
        nc.vector.memset(qaug_sb[64:128, :], 1.0)
        nc.vector.memset(kaug_sb[64:128, :], 1.0)

        # ---------------- DMAs ----------------
        # xt quarters on sync/gpsimd (projection-critical); weights then all
        # of xbd on scalar (xbd needed only at AV, ~25us in).
        nc.sync.dma_start(out=xt_sb[0][:], in_=xt[:, :, 0:256])
        nc.gpsimd.dma_start(out=xt_sb[1][:], in_=xt[:, :, 256:512])
        nc.sync.dma_start(out=xt_sb[2][:], in_=xt[:, :, 512:768])
        nc.gpsimd.dma_start(out=xt_sb[3][:], in_=xt[:, :, 768:1024])
        nc.scalar.dma_start(out=wtd_sb[:, :, 0:64], in_=wtd[:])
        nc.scalar.dma_start(out=wxd_sb[:, :, 0:64], in_=wxd[:])
        nc.scalar.dma_start(out=angw_sb[:], in_=angw[:])
        nc.scalar.dma_start(out=wamp_sb[:], in_=wamp[:])
        nc.scalar.dma_start(out=qsbm_sb[:], in_=qsbm[:])
        nc.scalar.dma_start(out=btcol_sb[:], in_=btcol[:])
        nc.scalar.dma_start(out=bac_sb[:], in_=bac[:])
        nc.scalar.dma_start(
            out=xb_sb[0][:], in_=xbd[:, 0, :]
        )
        for g in range(1, NG):
            nc.scalar.dma_start(out=xb_sb[g][:], in_=xbd[:, g, :])

        # weight dup: cols 64-127 = cols 0-63
        nc.vector.tensor_copy(wtd_sb[:, :, 64:128], wtd_sb[:, :, 0:64])
        nc.vector.tensor_copy(wxd_sb[:, :, 64:128], wxd_sb[:, :, 0:64])

        # PE warmup: start the HAM clock ramp during the xt DMA window
        warm_ctx = tc.tile_pool(name="warm_ps", bufs=1, space="PSUM")
        warm_pool = warm_ctx.__enter__()
        wt_ps = warm_pool.tile([128, 512], F32, tag="wt_ps")
        for _ in range(NWARM):
            nc.tensor.matmul(wt_ps[:], wdum_sb[:], ones1_sb[:],
                             start=True, stop=True)

        # ---------------- projections + features ----------------
        # qd/kd psums stay live through the feature phase: the STTs and the
        # m=0 Sins read them directly (no f32 psum->sbuf copies).
        feat_ctx = ExitStack()
        aqp = feat_ctx.enter_context(tc.tile_pool(name="aq_ps", bufs=2, space="PSUM"))
        akp = feat_ctx.enter_context(tc.tile_pool(name="ak_ps", bufs=1, space="PSUM"))
        rqp = feat_ctx.enter_context(tc.tile_pool(name="rq_sb", bufs=2))
        rkp = feat_ctx.enter_context(tc.tile_pool(name="rk_sb", bufs=2))
        with tc.tile_pool(name="proj_ps", bufs=1, space="PSUM") as pps:
            qd_ps = pps.tile([128, NQ], F32, tag="qd_ps")
            kd_ps = pps.tile([128, L], F32, tag="kd_ps")
            for qq in range(2):  # query half = quarters 0,1 (host-permuted)
                sl = slice(qq * 256, qq * 256 + 256)
                for c in range(DC):
                    nc.tensor.matmul(
                        qd_ps[:, sl], wtd_sb[:, c, :], xt_sb[qq][:, c, :],
                        start=(c == 0), stop=(c == DC - 1),
                    )
            nc.vector.tensor_scalar(
                qaug_sb[0:64, :], qd_ps[0:64, :], btcol_sb[0:64, 0:1], None,
                Alu.add,
            )
            nc.vector.tensor_scalar(
                qdup_sb[:], qd_ps[:], btcol_sb[:, 0:1], None, Alu.add
            )
            # m=0 Q feature straight off the projection psum
            nc.scalar.activation(
                qf_sb[:, 0, :], qd_ps[:], Act.Sin,
                bias=qsbm_sb[:, 0:1], scale=float(WS[0]),
            )
            nc.vector.tensor_scalar_mul(
                qfa_sb[:, 0, :], qf_sb[:, 0, :], wamp_sb[:, 0:1]
            )
            for qq in range(4):
                sl = slice(qq * 256, qq * 256 + 256)
                for c in range(DC):
                    nc.tensor.matmul(
                        kd_ps[:, sl], wxd_sb[:, c, :], xt_sb[qq][:, c, :],
                        start=(c == 0), stop=(c == DC - 1),
                    )
                if qq == 1:
                    nc.vector.tensor_copy(
                        kaug_sb[0:64, 0:512], kd_ps[0:64, 0:512]
                    )
                    nc.vector.tensor_copy(kdup_sb[:, 0:512], kd_ps[:, 0:512])
            nc.vector.tensor_copy(kaug_sb[0:64, 512:1024], kd_ps[0:64, 512:1024])
            nc.vector.tensor_copy(kdup_sb[:, 512:1024], kd_ps[:, 512:1024])
            nc.scalar.activation(
                kf_sb[:, 0, :], kd_ps[:], Act.Sin,
                bias=ksb_sb[:, 0:1], scale=float(WS[0]),
            )

        if True:
            for m in range(1, M):
                negp = float(-PS[m])
                w = float(WS[m])
                # Q side: 3-matmul rounding chain
                aq = aqp.tile([128, NQ], F32, tag="aq", name="aq")
                nc.tensor.matmul(aq[:], angw_sb[:, m - 1, :], qaug_sb[:],
                                 start=True, stop=False)
                nc.tensor.matmul(aq[:], angw_sb[:, 6, :], ones1_sb[:],
                                 start=False, stop=False)
                nc.tensor.matmul(aq[:], angw_sb[:, 7, :], ones1_sb[:],
                                 start=False, stop=True)
                rq = rqp.tile([128, NQ], F32, tag="rq", name="rq")
                nc.vector.scalar_tensor_tensor(
                    rq[:], aq[:], negp, qdup_sb[:], Alu.mult, Alu.add
                )
                nc.scalar.activation(
                    qf_sb[:, m, :], rq[:], Act.Sin,
                    bias=qsbm_sb[:, m:m + 1], scale=w,
                )
                nc.vector.tensor_scalar_mul(
                    qfa_sb[:, m, :], qf_sb[:, m, :], wamp_sb[:, m:m + 1]
                )
                # K side
                ak = akp.tile([128, L], F32, tag="ak", name="ak")
                for half in range(2):
                    sl = slice(half * 512, half * 512 + 512)
                    nc.tensor.matmul(ak[:, sl], angw_sb[:, 3 + m - 1, :],
                                     kaug_sb[:, sl], start=True, stop=False)
                    nc.tensor.matmul(ak[:, sl], angw_sb[:, 6, :], ones1_sb[:],
                                     start=False, stop=False)
                    nc.tensor.matmul(ak[:, sl], angw_sb[:, 7, :], ones1_sb[:],
                                     start=False, stop=True)
                rk = rkp.tile([128, L], F32, tag="rk", name="rk")
                nc.vector.scalar_tensor_tensor(
                    rk[:], ak[:], negp, kdup_sb[:], Alu.mult, Alu.add
                )
                nc.scalar.activation(
                    kf_sb[:, m, :], rk[:], Act.Sin,
                    bias=ksb_sb[:, 0:1], scale=w,
                )

        feat_ctx.close()
        warm_ctx.__exit__(None, None, None)

        # ---------------- scores / exp / AV ----------------
        sc_pool = ctx.enter_context(tc.tile_pool(name="sc", bufs=2, space="PSUM"))
        v_pool = ctx.enter_context(tc.tile_pool(name="vps", bufs=1, space="PSUM"))
        vo_pool = ctx.enter_context(tc.tile_pool(name="vo", bufs=1))
        v_tiles = [
            v_pool.tile([128, D], F32, tag=f"v{ic}", name=f"v{ic}")
            for ic in range(NI)
        ]
        den_ps = v_pool.tile([128, NI, 8], F32, tag="den")

        for g in range(NG):
            sc = sc_pool.tile([128, NQ], F32, tag="sc", name="sc")
            gsl = slice(g * 128, (g + 1) * 128)
            for m in range(M):
                nc.tensor.matmul(
                    sc[:], kf_sb[:, m, gsl], qfa_sb[:, m, :],
                    start=(m == 0), stop=(m == M - 1),
                )
            nc.scalar.activation(
                et_sb[:, g, :], sc[:], Act.Exp, bias=bac_sb[:, 0:1]
            )
            for ic in range(NI):
                isl = slice(ic * 128, (ic + 1) * 128)
                nc.tensor.matmul(
                    v_tiles[ic][:], et_sb[:, g, isl], xb_sb[g][:],
                    start=(g == 0), stop=(g == NG - 1),
                )
                nc.tensor.matmul(
                    den_ps[:, ic, :], et_sb[:, g, isl], onesd_sb[:],
                    start=(g == 0 and ic == 0),
                    stop=(g == NG - 1 and ic == NI - 1),
                )

        # ---------------- normalize + out ----------------
        rcol_sb = const.tile([128, NI], F32, tag="rcol")
        v_sb = vo_pool.tile([128, NI, D], F32, tag="vsb", name="v_sb")
        vout_r = vout.rearrange("(ic p) d -> p ic d", p=128)
        out_rings = (nc.sync, nc.gpsimd, nc.sync, nc.gpsimd)
        for ic in range(NI):
            nc.vector.reciprocal(rcol_sb[:, ic:ic + 1], den_ps[:, ic, 0:1])
            if ic % 2 == 0:
                nc.scalar.mul(v_sb[:, ic, :], v_tiles[ic][:],
                              rcol_sb[:, ic:ic + 1])
            else:
                nc.vector.tensor_scalar_mul(v_sb[:, ic, :], v_tiles[ic][:],
                                            rcol_sb[:, ic:ic + 1])
            out_rings[ic].dma_start(out=vout_r[:, ic:ic + 1, :],
                                    in_=v_sb[:, ic:ic + 1, :])

    nc.compile()
    _cached["nc"] = nc
    return nc


def _to_bf16(a):
    import ml_dtypes

    return np.asarray(a, dtype=np.float32).astype(ml_dtypes.bfloat16)


def _host_prep(x, Wx, Wt, bt, Wa, ba):
    x = np.ascontiguousarray(x, dtype=np.float32)
    Wx = np.asarray(Wx, dtype=np.float32)
    Wt = np.asarray(Wt, dtype=np.float32)
    bt = np.asarray(bt, dtype=np.float32).reshape(U)
    Wa = np.asarray(Wa, dtype=np.float32).reshape(U)
    ba = np.asarray(ba, dtype=np.float32).reshape(1)

    wtd = np.empty((128, DC, 64), dtype=np.float32)
    wxd = np.empty((128, DC, 64), dtype=np.float32)
    for c in range(DC):
        wtd[:, c, :] = Wt[128 * c:128 * (c + 1), :]
        wxd[:, c, :] = Wx[128 * c:128 * (c + 1), :]

    # angle stationaries: slots 0-2 wangq, 3-5 wangk, 6/7 = +/- MAGIC/128
    angs = np.zeros((128, 8, 128), dtype=np.float32)
    for m in range(1, M):
        invp = 1.0 / PS[m]
        for u in range(U):
            angs[u, m - 1, u] = invp
            angs[u, m - 1, 64 + u] = invp
            angs[u, 3 + m - 1, u] = invp
            angs[u, 3 + m - 1, 64 + u] = invp
        # Q offsets (qaug already carries q+bt): 1/4 turn on cos lanes
        angs[64, m - 1, 64:] = 0.25
        # K offsets: 1/4 on cos lanes (cols 0-63), 0 on sin lanes
        angs[64, 3 + m - 1, :64] = 0.25
    angs[:, 6, :] = MAGIC / 128
    angs[:, 7, :] = -MAGIC / 128

    wamp = np.empty((128, M), dtype=np.float32)
    qsb = np.empty((128, M), dtype=np.float32)
    for m in range(M):
        wamp[:64, m] = CS[m] * Wa
        wamp[64:, m] = CS[m] * Wa
        # m=0 Sin reads the raw projection psum (no bt) -> bias re-adds
        # w0*bt; m>=1 read qdup which already carries bt
        bt_term = WS[m] * bt if m == 0 else 0.0
        qsb[:64, m] = bt_term
        qsb[64:, m] = bt_term + HALF_PI
    btc = np.empty((128, 1), dtype=np.float32)
    btc[:64, 0] = bt
    btc[64:, 0] = bt
    bac = np.full((128, 1), ba[0], dtype=np.float32)

    shared = {
        "wtd": _to_bf16(wtd), "wxd": _to_bf16(wxd),
        "angw": _to_bf16(angs),
        "wamp": wamp, "qsbm": qsb, "btcol": btc, "bac": bac,
    }

    in_maps = []
    for cid in range(NCORES):
        b, h = cid // 2, cid % 2
        xT = x[b].T.reshape(DC, 128, L).transpose(1, 0, 2)  # [128, DC, L]
        xr = x[b]
        if h == 1:
            xT = np.concatenate([xT[:, :, 512:], xT[:, :, :512]], axis=2)
            xr = np.concatenate([xr[512:], xr[:512]], axis=0)
        xbv = xr.reshape(NG, 128, D).transpose(1, 0, 2)  # [128, NG, D]
        m_ = dict(shared)
        m_["xt"] = _to_bf16(np.ascontiguousarray(xT))
        m_["xbd"] = _to_bf16(np.ascontiguousarray(xbv))
        in_maps.append(m_)
    return in_maps


def kernel(x, Wx, Wt, bt, Wa, ba):
    nc = _build()
    in_maps = _host_prep(x, Wx, Wt, bt, Wa, ba)
    res = run_bass_kernel_spmd(nc, in_maps, core_ids=list(range(NCORES)))
    out = np.empty((B, L, D), dtype=np.float32)
    for cid in range(NCORES):
        b, h = cid // 2, cid % 2
        out[b, h * NQ:(h + 1) * NQ, :] = res.results[cid]["v_out"]
    return out


if __name__ == "__main__":
    rng = np.random.default_rng(0)
    x = rng.standard_normal((B, L, D), dtype=np.float32)
    Wx = (rng.standard_normal((D, U), dtype=np.float32) * 0.06).astype(np.float32)
    Wt = (rng.standard_normal((D, U), dtype=np.float32) * 0.06).astype(np.float32)
    bt = np.zeros(U, dtype=np.float32)
    Wa = (rng.standard_normal((U, 1), dtype=np.float32) * 0.17).astype(np.float32)
    ba = np.zeros(1, dtype=np.float32)
    v = kernel(x=x, Wx=Wx, Wt=Wt, bt=bt, Wa=Wa, ba=ba)
    print("kernel ran, out shape", v.shape)
